# revision 6
# baseline (speedup 1.0000x reference)
"""AlignmentLoss Trainium2 kernel v2.

Math (per sample b):
  z1h = z1 / max(||z1||, 1e-12);  z2h likewise        (L2 over D=768)
  For 25 shifts (dy,dx) in [-2,2]^2:
      sim_s[p] = <z1h[p], z2h[p+s]>   (zero outside the 96x96 grid)
  alpha = softmax(sim / (0.07+1e-10)) over the 25 shifts
  s*[p] = sum_s alpha_s sim_s
  align_loss = mean_b sum_p (1 - s*[p]) wbar[p],  wbar = 0.5*(w1/sum + w2/sum)
  pos_similarity = mean over all (b,p) of s*[p]

v2 strategy (vs v1): bf16 staging via casting SWDGE DMAs (bf16 transposes
run 1 cyc/row vs 2 for f32; PSUM bf16 tiles halve DVE copy cost), rsqrt as
exp(-0.5*ln(x)) so ScalarE stays in one act-func table (ln/exp/square/copy
coexist in natural_log_exp_and_others -> no 1.3us table reloads), r1 batched
per band, out-of-band masking folded into the Gram via an extra PE matmul
adding -1e30 (e underflows to 0) so B rides the Exp's accum_out and only one
DVE reduce (A) remains per block. Window cols 12 (bf16 matmul has no N>=256
constraint).

Layout (per band of 32 z1 rows):
  - z2 rows [z2y0, z2y0+36) transposed into 6 chunk slabs [128 d, 40*96]
    bf16, ghost rows zeroed; slab col of global y = (y - slab_row0)*96 + x.
  - r2rep [128, 3840] bf16 = 1/||z2|| per slab column, replicated to all
    partitions (Pool broadcast); g2 = G * r2rep window per block.
  - Per block (16x8 pixels): 6 accum matmuls (K=128) + 1 mask matmul give
    G[p, 20x12 window] + (-1e30 outside the 5x5 shift band); e = exp(g2*r1)
    with accum_out = B; A = sum g2*e*mask-free (amr); s* = A/B * r1 * T.
  - Out-of-image x handled by 3 mask variants + per-partition compensation
    counts added to B; out-of-image y by zeroed ghost rows (e = exp(0) = 1,
    matching the reference zero-padding).
"""

import os
import sys

for _p in ("/opt/trn_rl_repo",):
    if _p not in sys.path:
        sys.path.insert(0, _p)

import math
import numpy as np
from contextlib import ExitStack

import concourse.bass as bass
import concourse.bacc as bacc
import concourse.tile as tile
from concourse import mybir

F32 = mybir.dt.float32
BF16 = mybir.dt.bfloat16
I32 = mybir.dt.int32
ALU = mybir.AluOpType
ACTF = mybir.ActivationFunctionType

H = W = 96
D = 768
NPIX = H * W            # 9216
KC = D // 128           # 6 contraction chunks
TH, TW = 16, 8          # z1 pixel tile (128 partitions, p = 8*ty + tx)
WR, WC = 20, 12         # z2 window rows/cols
NWIN = WR * WC          # 240
NBX = W // TW           # 12 block cols
NBY_BAND = 2
BLOCKS_BAND = NBX * NBY_BAND  # 24
NBANDS = 3
SLAB_ROWS = 40
SLAB_FREE = SLAB_ROWS * W   # 3840
TEMP = 0.07 + 1e-10
LN_INV_T = -math.log(TEMP)
NEG = -1e30

# band i: z1 rows [32i, 32i+32); z2 loaded rows [z2y0, z2y0+36);
# slab row of global y = y - slab_row0 ; data col = (slabrow+1)*96 + x
# (the +1 guard row absorbs the woff-2 underflow of bx=0 blocks)
_Z2Y0 = (0, 30, 60)
_SLAB_ROW0 = (-2, 30, 60)
# memset these (start, size) elem ranges of each slab; DMA fills the rest
_MEMSET = (
    ((0, 3 * W), (39 * W, W)),          # guard row + 2 ghost rows ; pad row
    ((0, W), (37 * W, 3 * W)),          # guard row ; rows 36,37 unused + pad
    ((0, W), (37 * W, 3 * W)),          # guard row ; 2 ghost rows + pad
)
_LOAD_COL0 = (3 * W, W, W)              # col of first loaded pixel

NT_Z2 = 27              # z2 tiles of 128 pixels per band (27*128 = 3456 px)
Z2G = 4                 # tiles per z2 load group


def _sub(ap, off, dims):
    """AP at element offset `off` into `ap`'s free space with free dims `dims`."""
    return bass.AP(ap.tensor, ap.offset + off, [list(ap.ap[0])] + [list(d) for d in dims])


def _split_pe_waits(nc):
    """Walrus codegen accepts at most one sync-wait per compute-engine
    instruction: hoist excess waits onto wait-carrier NoOps."""
    n = 0
    for f in nc.m.functions:
        for blk in f.blocks:
            out = []
            for ins in blk.instructions:
                si = ins.sync_info
                if si is not None and si.on_wait and len(si.on_wait) > 1:
                    for w in si.on_wait[:-1]:
                        n += 1
                        out.append(mybir.InstNoOp(
                            name=f"EVW-{n}-{ins.name}",
                            engine=ins.engine,
                            text_hint="wait_carrier",
                            bass_nofuse=True,
                            ins=[], outs=[],
                            sync_info=mybir.SyncInfo(on_wait=[w], on_update=[]),
                        ))
                    ins.sync_info = mybir.SyncInfo(
                        on_wait=[si.on_wait[-1]], on_update=si.on_update)
                out.append(ins)
            blk.instructions = out


def _emit(ctx: ExitStack, tc: tile.TileContext, z1, z2, w1, w2, out):
    nc = tc.nc
    KST = int(os.environ.get("KST", "9"))
    CAST = int(os.environ.get("KCAST", "1"))
    PS32 = int(os.environ.get("KPS32", "0"))
    KSQ = int(os.environ.get("KSQ", "1"))
    KGD = int(os.environ.get("KGD", "1"))

    const = ctx.enter_context(tc.tile_pool(name="const", bufs=1))
    btmp_ctx = ExitStack()
    btmp = btmp_ctx.enter_context(tc.tile_pool(name="btmp", bufs=1))

    # ---------------- constants ----------------
    jj_i = btmp.tile([128, 128], I32)
    nc.gpsimd.iota(jj_i, pattern=[[1, 128]], base=0, channel_multiplier=0)
    jj_f = btmp.tile([128, 128], F32)
    nc.vector.tensor_copy(jj_f, jj_i)
    pv0 = btmp.tile([128, 1], I32)
    nc.gpsimd.iota(pv0, pattern=[[1, 1]], base=0, channel_multiplier=1)
    pv0f = btmp.tile([128, 1], F32)
    nc.vector.tensor_copy(pv0f, pv0)
    ident32 = const.tile([128, 128], F32)
    nc.vector.tensor_scalar(ident32, jj_f, pv0f, None, op0=ALU.is_equal)
    ident_bf = const.tile([128, 128], BF16)
    nc.vector.tensor_copy(ident_bf, ident32)

    ones_c = const.tile([128, 1], F32)
    nc.vector.memset(ones_c, 1.0)
    zsrc = const.tile([128, 3 * W], F32)
    nc.vector.memset(zsrc, 0.0)
    onesrc = const.tile([128, 3 * W], F32)
    nc.vector.memset(onesrc, 1.0)
    ones_r = const.tile([1, 128], F32)
    nc.vector.memset(ones_r, 1.0)
    zb_bf = const.tile([128, 1], BF16)
    nc.vector.memset(zb_bf, 0.0)
    warm = const.tile([128, 1], BF16)
    nc.scalar.activation(warm, zb_bf, ACTF.Square, bias=zb_bf)

    # band masks over the 20x12 window: in-band iff (0<=ny-ty<5)(0<=nx-tx<5)
    # (ty=p>>3, tx=p&7). maskc_* = 0 in-band else -1e30 (added to the Gram);
    # m_* = 1 in-band else 0 (bf16, for the A reduce).
    tyv_i = btmp.tile([128, 1], I32)
    nc.vector.tensor_scalar(tyv_i, pv0, 3, None, op0=ALU.logical_shift_right)
    txv_i = btmp.tile([128, 1], I32)
    nc.vector.tensor_scalar(txv_i, pv0, 7, None, op0=ALU.bitwise_and)
    tyv = btmp.tile([128, 1], F32)
    nc.vector.tensor_copy(tyv, tyv_i)
    txv = btmp.tile([128, 1], F32)
    nc.vector.tensor_copy(txv, txv_i)

    nyt_i = btmp.tile([128, NWIN], I32)
    nc.gpsimd.iota(nyt_i.rearrange("p (a b) -> p a b", b=WC),
                   pattern=[[1, WR], [0, WC]], base=0, channel_multiplier=0)
    nxt_i = btmp.tile([128, NWIN], I32)
    nc.gpsimd.iota(nxt_i.rearrange("p (a b) -> p a b", b=WC),
                   pattern=[[0, WR], [1, WC]], base=0, channel_multiplier=0)
    nyt = btmp.tile([128, NWIN], F32)
    nc.vector.tensor_copy(nyt, nyt_i)
    nxt = btmp.tile([128, NWIN], F32)
    nc.vector.tensor_copy(nxt, nxt_i)

    dyt = btmp.tile([128, NWIN], F32)
    nc.vector.tensor_scalar(dyt, nyt, tyv, None, op0=ALU.subtract)
    dxt = btmp.tile([128, NWIN], F32)
    nc.vector.tensor_scalar(dxt, nxt, txv, None, op0=ALU.subtract)

    tmp_a = btmp.tile([128, NWIN], F32)
    tmp_b = btmp.tile([128, NWIN], F32)
    mf = btmp.tile([128, NWIN], F32)
    nc.vector.tensor_scalar(tmp_a, dyt, 0.0, None, op0=ALU.is_ge)
    nc.vector.tensor_scalar(tmp_b, dyt, 5.0, None, op0=ALU.is_lt)
    nc.vector.tensor_mul(mf, tmp_a, tmp_b)
    nc.vector.tensor_scalar(tmp_a, dxt, 0.0, None, op0=ALU.is_ge)
    nc.vector.tensor_mul(mf, mf, tmp_a)
    nc.vector.tensor_scalar(tmp_b, dxt, 5.0, None, op0=ALU.is_lt)
    nc.vector.tensor_mul(mf, mf, tmp_b)

    mf_l = btmp.tile([128, NWIN], F32)
    nc.vector.tensor_scalar(tmp_a, nxt, 2.0, None, op0=ALU.is_ge)
    nc.vector.tensor_mul(mf_l, mf, tmp_a)
    mf_r = btmp.tile([128, NWIN], F32)
    nc.vector.tensor_scalar(tmp_b, nxt, 9.0, None, op0=ALU.is_le)
    nc.vector.tensor_mul(mf_r, mf, tmp_b)

    maskc_int = const.tile([128, NWIN], BF16)
    maskc_left = const.tile([128, NWIN], BF16)
    maskc_right = const.tile([128, NWIN], BF16)
    for mcb, src in ((maskc_int, mf), (maskc_left, mf_l), (maskc_right, mf_r)):
        # maskc = (m - 1) * 1e30  -> 0 in-band, -1e30 out
        nc.vector.tensor_scalar(tmp_a, src, -1.0, -NEG, op0=ALU.add, op1=ALU.mult)
        nc.vector.tensor_copy(mcb, tmp_a)

    # B compensation: +1 per out-of-image (dy,dx) the x-masks removed
    cl_f = btmp.tile([128, 1], F32)
    nc.vector.tensor_scalar(cl_f, txv, -1.0, 2.0, op0=ALU.mult, op1=ALU.add)  # 2-tx
    c_left = btmp.tile([128, 1], F32)
    nc.vector.tensor_scalar(c_left, cl_f, 0.0, 5.0, op0=ALU.max, op1=ALU.mult)
    cr_f = btmp.tile([128, 1], F32)
    nc.vector.tensor_scalar(cr_f, txv, 5.0, None, op0=ALU.subtract)           # tx-5
    c_right = btmp.tile([128, 1], F32)
    nc.vector.tensor_scalar(c_right, cr_f, 0.0, 5.0, op0=ALU.max, op1=ALU.mult)

    ctile = const.tile([128, BLOCKS_BAND], F32)
    nc.vector.memset(ctile, 0.0)
    for cc, src in ((0, c_left), (12, c_left), (11, c_right), (23, c_right)):
        nc.vector.tensor_copy(ctile[:, cc:cc + 1], src)

    btmp_ctx.close()

    CAST0 = int(os.environ.get("KCAST", "1"))
    cols = ctx.enter_context(tc.tile_pool(name="cols", bufs=1))
    colsb = ctx.enter_context(tc.tile_pool(name="colsb", bufs=2))
    slabp = ctx.enter_context(tc.tile_pool(name="slab", bufs=2 if CAST0 else 1))
    z2stage = ctx.enter_context(tc.tile_pool(name="z2stage", bufs=3 if CAST0 else 2))
    z1stage = ctx.enter_context(tc.tile_pool(name="z1stage", bufs=8 if CAST0 else 4))
    sqscr = ctx.enter_context(tc.tile_pool(name="sqscr", bufs=2))
    z1tp = ctx.enter_context(tc.tile_pool(name="z1tp", bufs=1))
    gstage = ctx.enter_context(tc.tile_pool(name="gstage", bufs=4))
    psum_tp = ctx.enter_context(tc.tile_pool(name="psum_tp", bufs=2, space="PSUM"))
    psum_g = ctx.enter_context(tc.tile_pool(name="psum_g", bufs=3, space="PSUM"))
    psum_s = ctx.enter_context(tc.tile_pool(name="psum_s", bufs=1, space="PSUM"))

    # ---------------- w_avg ----------------
    wt1 = const.tile([128, 72], F32)
    wt2 = const.tile([128, 72], F32)
    for wt, wsrc in ((wt1, w1), (wt2, w2)):
        nc.sync.dma_start(out=wt, in_=wsrc)
    wbar = const.tile([128, 72], F32)
    winv = const.tile([128, 2], F32)
    for j, wt in enumerate((wt1, wt2)):
        wsum = cols.tile([128, 1], F32, name=f"wsum{j}")
        nc.vector.reduce_sum(wsum, wt, axis=mybir.AxisListType.X)
        pswt = psum_s.tile([1, 1], F32, name=f"pswt{j}", tag="s")
        nc.tensor.matmul(pswt, lhsT=wsum, rhs=ones_c, start=True, stop=True)
        stot = cols.tile([1, 1], F32, name=f"stot{j}")
        nc.scalar.copy(stot, pswt)
        nc.vector.tensor_scalar(stot, stot, 1e-10, None, op0=ALU.add)
        nc.vector.reciprocal(stot, stot)
        psb = psum_s.tile([128, 1], F32, name=f"psb{j}", tag="s")
        nc.tensor.matmul(psb, lhsT=ones_r, rhs=stot, start=True, stop=True)
        nc.scalar.copy(winv[:, j:j + 1], psb)
    t_a = const.tile([128, 72], F32)
    nc.vector.tensor_scalar_mul(t_a, wt1, winv[:, 0:1])
    t_b = const.tile([128, 72], F32)
    nc.vector.tensor_scalar_mul(t_b, wt2, winv[:, 1:2])
    nc.vector.tensor_add(wbar, t_a, t_b)
    nc.vector.tensor_scalar_mul(wbar, wbar, 0.5)

    alignB = cols.tile([128, NBANDS], F32)
    posB = cols.tile([128, NBANDS], F32)
    nc.vector.memset(alignB, 0.0)
    nc.vector.memset(posB, 0.0)

    # ---------------- band state ----------------
    state = {}

    def emit_rsqrt(ss, y, n, tag, inv_scale=1.0):
        """y[:, :n] = inv_scale / sqrt(ss[:, :n]) via quake seed + 3 Newton
        iterations, all on DVE (no ScalarE act-table traffic)."""
        qi = colsb.tile([128, ss.shape[1]], I32, name=f"q_{tag}", tag=f"q_{tag}")
        nc.vector.tensor_scalar(qi[:, :n], ss[:, :n].bitcast(I32), 1, None,
                                op0=ALU.logical_shift_right)
        nc.vector.tensor_scalar(qi[:, :n], qi[:, :n], -1, None,
                                op0=ALU.bitwise_xor)
        nc.vector.tensor_scalar(qi[:, :n], qi[:, :n], 0x5F3759E0, None,
                                op0=ALU.add)
        yv = y[:, :n]
        t1 = colsb.tile([128, ss.shape[1]], F32, name=f"t1_{tag}", tag=f"t1_{tag}")
        t2 = colsb.tile([128, ss.shape[1]], F32, name=f"t2_{tag}", tag=f"t2_{tag}")
        cur = qi.bitcast(F32)[:, :n]
        for it in range(2):
            last = it == 1
            nc.vector.tensor_mul(t1[:, :n], ss[:, :n], cur)
            nc.vector.tensor_mul(t2[:, :n], t1[:, :n], cur)
            sc = inv_scale if last else 1.0
            nc.vector.tensor_scalar(t2[:, :n], t2[:, :n], -0.5 * sc, 1.5 * sc,
                                    op0=ALU.mult, op1=ALU.add)
            nc.vector.tensor_mul(yv, cur, t2[:, :n])
            cur = yv
        return y

    def emit_A_start(band):
        """Allocate band tiles, memset slab ghosts."""
        z2y0 = _Z2Y0[band]
        slab6 = slabp.tile([128, KC * SLAB_FREE], BF16, name=f"slab6_{band}",
                           tag="slab6")
        r2rep = slabp.tile([128, SLAB_FREE], BF16, name=f"r2rep{band}", tag="r2rep")
        r2band = colsb.tile([1, NT_Z2 * 128], BF16, name=f"r2band{band}", tag="r2band")
        ss2 = colsb.tile([128, NT_Z2], F32, name=f"ss2_{band}", tag="ss2")
        ss1 = colsb.tile([128, BLOCKS_BAND], F32, name=f"ss1_{band}", tag="ss1")
        for rng in _MEMSET[band]:
            nc.gpsimd.dma_start(
                out=_sub(slab6, rng[0], [[SLAB_FREE, KC], [1, rng[1]]]),
                in_=bass.AP(zsrc.tensor, zsrc.offset,
                            [list(zsrc.ap[0]), [0, KC], [1, rng[1]]]))
            # ghost/guard cols of r2rep must be 1.0: in-band ghost slots need
            # g2 = 0*1 = 0 (e=1, matching zero-padding), out-of-band ghost
            # slots need g2 = -1e30*1 so e underflows to 0.
            nc.gpsimd.dma_start(
                out=r2rep[:, rng[0]:rng[0] + rng[1]],
                in_=onesrc[:, 0:rng[1]])
        st = dict(slab6=slab6, r2rep=r2rep, r2band=r2band, ss2=ss2, ss1=ss1,
                  z2y0=z2y0, slab_row0=_SLAB_ROW0[band],
                  z1T=[None] * BLOCKS_BAND, r1T=None, acol=None, bcol=None)
        state[band] = st
        return st

    def emit_z2_group(band, grp):
        """Load z2 tiles (cast to bf16), square, transpose into the slab."""
        st = state[band]
        t0, gsz = grp
        gi = t0
        z2y0 = st["z2y0"]
        zt = z2stage.tile([128, gsz * D], BF16, name=f"z2t{band}_{gi}", tag="z2t")
        r0 = z2y0 * W + t0 * 128
        if CAST and KGD:
            # one casting SWDGE DMA for the whole contiguous group
            nc.gpsimd.dma_start(
                out=zt.rearrange("p (t c) -> p t c", c=D),
                in_=bass.AP(z2.tensor, r0 * D,
                            [[D, 128], [128 * D, gsz], [1, D]]))
        elif CAST:
            for j in range(gsz):
                nc.gpsimd.dma_start(
                    out=zt[:, j * D:(j + 1) * D],
                    in_=bass.AP(z2.tensor, (r0 + j * 128) * D,
                                [[D, 128], [1, D]]))
        else:
            ztf = z2stage.tile([128, gsz * D], F32, name=f"z2f{band}_{gi}",
                               tag="z2f")
            st["_ztf"] = ztf
            nc.sync.dma_start(
                out=ztf.rearrange("p (t c) -> p t c", c=D),
                in_=bass.AP(z2.tensor, r0 * D,
                            [[D, 128], [128 * D, gsz], [1, D]]))
            nc.vector.tensor_copy(zt, ztf)
        if PS32:
            assert not CAST
        dst = _LOAD_COL0[band] + t0 * 128
        for j in range(gsz if KSQ else 0):
            t = t0 + j
            if t % 3 == 0 or t == 26:
                continue  # norm via PE diag below (after slab copies)
            scr = sqscr.tile([128, D], BF16, name=f"z2sq{band}_{t}", tag="zsq")
            nc.scalar.activation(scr, zt[:, j * D:(j + 1) * D], ACTF.Square,
                                 bias=zb_bf, accum_out=st["ss2"][:, t:t + 1])
        # transposes into slab6
        for k in range(KC):
            if PS32:
                ps = psum_tp.tile([128, Z2G * 128], F32,
                                  name=f"z2ps{band}_{gi}_{k}", tag="tp")
                for j in range(gsz):
                    nc.tensor.transpose(
                        ps[:, j * 128:(j + 1) * 128],
                        ztf[:, j * D + k * 128:j * D + (k + 1) * 128],
                        ident32)
            else:
                ps = psum_tp.tile([128, Z2G * 128], BF16,
                                  name=f"z2ps{band}_{gi}_{k}", tag="tp")
                for j in range(gsz):
                    nc.tensor.transpose(
                        ps[:, j * 128:(j + 1) * 128],
                        zt[:, j * D + k * 128:j * D + (k + 1) * 128],
                        ident_bf)
            nc.vector.tensor_copy(
                _sub(st["slab6"], k * SLAB_FREE + dst, [[1, gsz * 128]]),
                ps[:, :gsz * 128])
        # z2 norms for selected tiles: PE self-matmul Gram diagonal
        for j in range(gsz if KSQ else 0):
            t = t0 + j
            if not (t % 3 == 0 or t == 26):
                continue
            sg = psum_s.tile([128, 128], F32, name=f"sg{band}_{t}", tag="sg",
                             bufs=1)
            for k in range(KC):
                cols = _sub(st["slab6"], k * SLAB_FREE + dst + j * 128,
                            [[1, 128]])
                nc.tensor.matmul(sg, lhsT=cols, rhs=cols,
                                 start=(k == 0), stop=(k == KC - 1))
            dscr = gstage.tile([128, 128], F32, name=f"dsc{band}_{t}",
                               tag="dscr", bufs=2)
            nc.vector.affine_mul_reduce(
                out=dscr, accum_out=st["ss2"][:, t:t + 1],
                in0=sg, in1=ident32, scale=1.0, bias=0.0)

    def emit_z1_block_load(band, bb):
        """Load z1 block (cast), square (DVE), transpose into z1T."""
        st = state[band]
        byl, bx = divmod(bb, NBX)
        y0 = 32 * band + TH * byl
        z1t = z1stage.tile([128, D], BF16, name=f"z1t{band}_{bb}", tag="z1t")
        if CAST:
            nc.gpsimd.dma_start(
                out=z1t,
                in_=bass.AP(z1.tensor, (y0 * W + bx * TW) * D,
                            [[W * D, TH], [D, TW], [1, D]]))
        else:
            z1f = z1stage.tile([128, D], F32, name=f"z1f{band}_{bb}", tag="z1f")
            nc.sync.dma_start(
                out=z1f,
                in_=bass.AP(z1.tensor, (y0 * W + bx * TW) * D,
                            [[W * D, TH], [D, TW], [1, D]]))
            nc.vector.tensor_copy(z1t, z1f)
        if KSQ and bb % 8 != 0:
            scr = sqscr.tile([128, D], BF16, name=f"z1sq{band}_{bb}", tag="zsq")
            nc.scalar.activation(scr, z1t, ACTF.Square, bias=zb_bf,
                                 accum_out=st["ss1"][:, bb:bb + 1])
        z1T = z1tp.tile([128, D], BF16, name=f"z1T{band}_{bb}", tag=f"z1T{bb}")
        tsrc, tid = (z1f, ident32) if PS32 else (z1t, ident_bf)
        tdt = F32 if PS32 else BF16
        psa = psum_tp.tile([128, D], tdt, name=f"z1psa{band}_{bb}", tag="tpz", bufs=1)
        for k in range(KC):
            nc.tensor.transpose(psa[:, k * 128:(k + 1) * 128],
                                tsrc[:, k * 128:(k + 1) * 128], tid)
        nc.vector.tensor_copy(z1T, psa)
        st["z1T"][bb] = z1T
        if KSQ and bb % 8 == 0:
            sg = psum_s.tile([128, 128], F32, name=f"sg1_{band}_{bb}", tag="sg",
                             bufs=1)
            for k in range(KC):
                nc.tensor.matmul(sg, lhsT=z1T[:, k * 128:(k + 1) * 128],
                                 rhs=z1T[:, k * 128:(k + 1) * 128],
                                 start=(k == 0), stop=(k == KC - 1))
            dscr = gstage.tile([128, 128], F32, name=f"dsc1_{band}_{bb}",
                               tag="dscr", bufs=2)
            nc.vector.affine_mul_reduce(
                out=dscr, accum_out=st["ss1"][:, bb:bb + 1],
                in0=sg, in1=ident32, scale=1.0, bias=0.0)

    def emit_r2(band):
        """Band-level r2 = 1/||z2|| -> r2band row -> broadcast into r2rep."""
        if KST < 2:
            return
        st = state[band]
        r2f = colsb.tile([128, NT_Z2], F32, name=f"r2f{band}", tag="r2f")
        emit_rsqrt(st["ss2"], r2f, NT_Z2, "r2")
        r2c = colsb.tile([128, NT_Z2], BF16, name=f"r2c_{band}", tag="r2c")
        nc.vector.tensor_copy(r2c, r2f)
        psr = psum_s.tile([NT_Z2, 128], BF16, name=f"psr2_{band}", tag="s")
        nc.tensor.transpose(psr, r2c, ident_bf)
        r2t = cols.tile([NT_Z2, 128], BF16, name=f"r2t{band}", tag="r2t", bufs=2)
        nc.vector.tensor_copy(r2t, psr)
        nc.sync.dma_start(
            out=_sub(st["r2band"], 0, [[1, NT_Z2 * 128]]), in_=r2t)
        nc.gpsimd.partition_broadcast(
            _sub(st["r2rep"], _LOAD_COL0[band], [[1, NT_Z2 * 128]]),
            _sub(st["r2band"], 0, [[1, NT_Z2 * 128]]))

    def emit_B(band):
        """Batched r1 = 1/(||z1||*T) via DVE quake rsqrt."""
        st = state[band]
        r1T = colsb.tile([128, BLOCKS_BAND], F32, name=f"r1T{band}", tag="r1T")
        if KST >= 2:
            emit_rsqrt(st["ss1"], r1T, BLOCKS_BAND, "r1", inv_scale=1.0 / TEMP)
        else:
            nc.vector.memset(r1T, 1.0)
        r1b = colsb.tile([128, BLOCKS_BAND], BF16, name=f"r1b{band}", tag="r1b")
        nc.vector.tensor_copy(r1b, r1T)
        st["r1T"] = r1T
        st["r1b"] = r1b
        st["acol"] = colsb.tile([128, BLOCKS_BAND], F32, name=f"acol{band}", tag="acol")
        st["bcol"] = colsb.tile([128, BLOCKS_BAND], F32, name=f"bcol{band}", tag="bcol")

    def emit_C_block(band, bb):
        st = state[band]
        byl, bx = divmod(bb, NBX)
        y0 = 32 * band + TH * byl
        s0 = (y0 - 2) - st["slab_row0"]
        woff = (s0 + 1) * W + TW * bx - 2
        maskc = maskc_left if bx == 0 else (maskc_right if bx == NBX - 1 else maskc_int)

        if KST < 3:
            return
        g = psum_g.tile([128, NWIN], F32, name=f"g{band}_{bb}", tag="g")
        nc.tensor.matmul(g, lhsT=ident_bf, rhs=maskc, start=True, stop=False)
        z1T = st["z1T"][bb]
        for k in range(KC):
            nc.tensor.matmul(
                g,
                lhsT=z1T[:, k * 128:(k + 1) * 128],
                rhs=_sub(st["slab6"], k * SLAB_FREE + woff, [[W, WR], [1, WC]]),
                start=False, stop=(k == KC - 1))

        if KST < 4:
            return
        g2 = gstage.tile([128, NWIN], BF16, name=f"g2_{band}_{bb}", tag="g2")
        nc.vector.tensor_mul(g2, g, _sub(st["r2rep"], woff, [[W, WR], [1, WC]]))
        e_t = gstage.tile([128, NWIN], BF16, name=f"e{band}_{bb}", tag="e")
        nc.scalar.activation(e_t, g2, ACTF.Exp, bias=zb_bf,
                             scale=st["r1T"][:, bb:bb + 1],
                             accum_out=st["bcol"][:, bb:bb + 1])
        scr2 = gstage.tile([128, NWIN], BF16, name=f"tt2s{band}_{bb}", tag="tt2s")
        nc.vector.affine_mul_reduce(
            out=scr2, accum_out=st["acol"][:, bb:bb + 1],
            in0=g2, in1=e_t, scale=1.0, bias=0.0)

    def emit_tail(band):
        if KST < 5:
            return
        st = state[band]
        bcol, acol, r1T = st["bcol"], st["acol"], st["r1T"]
        nc.vector.tensor_add(bcol, bcol, ctile)
        rB = colsb.tile([128, BLOCKS_BAND], F32, name=f"rB{band}", tag="rB")
        nc.vector.reciprocal(rB, bcol)
        sstar = colsb.tile([128, BLOCKS_BAND], F32, name=f"sstar{band}", tag="sstar")
        nc.vector.tensor_mul(sstar, acol, r1T)
        nc.vector.tensor_mul(sstar, sstar, rB)
        nc.vector.tensor_scalar_mul(sstar, sstar, TEMP)

        u = colsb.tile([128, BLOCKS_BAND], F32, name=f"u{band}", tag="u")
        nc.vector.tensor_scalar(u, sstar, -1.0, 1.0, op0=ALU.mult, op1=ALU.add)
        uw = colsb.tile([128, BLOCKS_BAND], F32, name=f"uw{band}", tag="uw")
        nc.vector.affine_mul_reduce(
            out=uw, accum_out=alignB[:, band:band + 1],
            in0=u, in1=wbar[:, band * 24:(band + 1) * 24],
            scale=1.0, bias=0.0)
        nc.vector.reduce_sum(posB[:, band:band + 1], sstar,
                             axis=mybir.AxisListType.X)

    # ---------------- schedule ----------------
    # A-items per band: 7 z2 groups + 24 z1 block loads, interleaved so DMA
    # and the square/transpose engines stay fed. Bands are software-pipelined:
    # band b+1's A-items are emitted between band b's C-blocks.
    def a_items(band):
        sizes = (4, 4, 4, 4, 4, 4, 3)
        items = []
        bi = 0
        t0 = 0
        for gi, gsz in enumerate(sizes):
            items.append(("g", (t0, gsz)))
            t0 += gsz
            if gi == len(sizes) - 1:
                items.append(("r2", 0))
            for _ in range(3):
                if bi < BLOCKS_BAND:
                    items.append(("b", bi))
                    bi += 1
        while bi < BLOCKS_BAND:
            items.append(("b", bi))
            bi += 1
        return items

    def emit_a_item(band, it):
        if it[0] == "g":
            emit_z2_group(band, it[1])
        elif it[0] == "r2":
            emit_r2(band)
        else:
            emit_z1_block_load(band, it[1])

    emit_A_start(0)
    for it in a_items(0):
        emit_a_item(0, it)
    emit_B(0)
    for band in range(NBANDS):
        nxt = band + 1
        if nxt < NBANDS:
            emit_A_start(nxt)
            items = a_items(nxt)
            # ~31 A-items over 24 C-blocks
            k = 0
            for bb in range(BLOCKS_BAND):
                emit_C_block(band, bb)
                want = ((bb + 1) * len(items) + BLOCKS_BAND - 1) // BLOCKS_BAND
                while k < min(want, len(items)):
                    emit_a_item(nxt, items[k])
                    k += 1
            while k < len(items):
                emit_a_item(nxt, items[k])
                k += 1
            emit_tail(band)
            emit_B(nxt)
        else:
            for bb in range(BLOCKS_BAND):
                emit_C_block(band, bb)
            emit_tail(band)

    # ---------------- final scalars ----------------
    if KST < 5:
        nc.vector.memset(alignB, 0.0)
        nc.vector.memset(posB, 0.0)
    fin = cols.tile([128, 2], F32)
    nc.vector.reduce_sum(fin[:, 0:1], alignB, axis=mybir.AxisListType.X)
    nc.vector.reduce_sum(fin[:, 1:2], posB, axis=mybir.AxisListType.X)
    psf = psum_s.tile([2, 1], F32, tag="s")
    nc.tensor.matmul(psf, lhsT=fin, rhs=ones_c, start=True, stop=True)
    fsb = cols.tile([2, 1], F32)
    nc.scalar.copy(fsb, psf)
    nc.sync.dma_start(out=out[:, :], in_=fsb)


def build_nc():
    nc = bacc.Bacc("TRN2", target_bir_lowering=False, debug=False)
    z1 = nc.dram_tensor("z1", [NPIX, D], F32, kind="ExternalInput")
    z2 = nc.dram_tensor("z2", [NPIX, D], F32, kind="ExternalInput")
    w1 = nc.dram_tensor("w1", [128, 72], F32, kind="ExternalInput")
    w2 = nc.dram_tensor("w2", [128, 72], F32, kind="ExternalInput")
    out = nc.dram_tensor("out", [2, 1], F32, kind="ExternalOutput")
    with tile.TileContext(nc) as tc:
        with ExitStack() as ctx:
            _emit(ctx, tc, z1.ap(), z2.ap(), w1.ap(), w2.ap(), out.ap())
    nc.compile()
    return nc


_NC_CACHE = None


def _get_nc():
    global _NC_CACHE
    if _NC_CACHE is None:
        _NC_CACHE = build_nc()
    return _NC_CACHE


def make_in_maps(z1, z2, w1, w2):
    z1 = np.asarray(z1, dtype=np.float32)
    z2 = np.asarray(z2, dtype=np.float32)

    def _wblk(w):
        w = np.asarray(w, dtype=np.float32).reshape(z1.shape[0], 6, TH, NBX, TW)
        return np.ascontiguousarray(
            w.transpose(0, 2, 4, 1, 3).reshape(z1.shape[0], 128, 72))
    w1 = _wblk(w1)
    w2 = _wblk(w2)
    return [
        {
            "z1": np.ascontiguousarray(z1[b]),
            "z2": np.ascontiguousarray(z2[b]),
            "w1": np.ascontiguousarray(w1[b]),
            "w2": np.ascontiguousarray(w2[b]),
        }
        for b in range(z1.shape[0])
    ]


def combine_outputs(outs):
    a = np.stack([o.reshape(2) for o in outs])  # [B, 2]
    align = a[:, 0].mean()
    pos = (a[:, 1] / NPIX).mean()
    return np.stack([align, pos]).astype(np.float32)


def kernel(z1, z2, w1, w2, H=None, W=None, **_unused):
    from concourse.bass_utils import run_bass_kernel_spmd

    nc = _get_nc()
    in_maps = make_in_maps(z1, z2, w1, w2)
    res = run_bass_kernel_spmd(nc, in_maps, core_ids=list(range(len(in_maps))))
    return combine_outputs([r["out"] for r in res.results])


# revision 7
# speedup vs baseline: 1.2534x; 1.2534x over previous
"""AlignmentLoss Trainium2 kernel v2.

Math (per sample b):
  z1h = z1 / max(||z1||, 1e-12);  z2h likewise        (L2 over D=768)
  For 25 shifts (dy,dx) in [-2,2]^2:
      sim_s[p] = <z1h[p], z2h[p+s]>   (zero outside the 96x96 grid)
  alpha = softmax(sim / (0.07+1e-10)) over the 25 shifts
  s*[p] = sum_s alpha_s sim_s
  align_loss = mean_b sum_p (1 - s*[p]) wbar[p],  wbar = 0.5*(w1/sum + w2/sum)
  pos_similarity = mean over all (b,p) of s*[p]

v2 strategy (vs v1): bf16 staging via casting SWDGE DMAs (bf16 transposes
run 1 cyc/row vs 2 for f32; PSUM bf16 tiles halve DVE copy cost), rsqrt as
exp(-0.5*ln(x)) so ScalarE stays in one act-func table (ln/exp/square/copy
coexist in natural_log_exp_and_others -> no 1.3us table reloads), r1 batched
per band, out-of-band masking folded into the Gram via an extra PE matmul
adding -1e30 (e underflows to 0) so B rides the Exp's accum_out and only one
DVE reduce (A) remains per block. Window cols 12 (bf16 matmul has no N>=256
constraint).

Layout (per band of 32 z1 rows):
  - z2 rows [z2y0, z2y0+36) transposed into 6 chunk slabs [128 d, 40*96]
    bf16, ghost rows zeroed; slab col of global y = (y - slab_row0)*96 + x.
  - r2rep [128, 3840] bf16 = 1/||z2|| per slab column, replicated to all
    partitions (Pool broadcast); g2 = G * r2rep window per block.
  - Per block (16x8 pixels): 6 accum matmuls (K=128) + 1 mask matmul give
    G[p, 20x12 window] + (-1e30 outside the 5x5 shift band); e = exp(g2*r1)
    with accum_out = B; A = sum g2*e*mask-free (amr); s* = A/B * r1 * T.
  - Out-of-image x handled by 3 mask variants + per-partition compensation
    counts added to B; out-of-image y by zeroed ghost rows (e = exp(0) = 1,
    matching the reference zero-padding).
"""

import os
import sys

for _p in ("/opt/trn_rl_repo",):
    if _p not in sys.path:
        sys.path.insert(0, _p)

import math
import numpy as np
from contextlib import ExitStack

import concourse.bass as bass
import concourse.bacc as bacc
import concourse.tile as tile
from concourse import mybir

F32 = mybir.dt.float32
BF16 = mybir.dt.bfloat16
I32 = mybir.dt.int32
ALU = mybir.AluOpType
ACTF = mybir.ActivationFunctionType

H = W = 96
D = 768
NPIX = H * W            # 9216
KC = D // 128           # 6 contraction chunks
TH, TW = 16, 8          # z1 pixel tile (128 partitions, p = 8*ty + tx)
WR, WC = 20, 12         # z2 window rows/cols
NWIN = WR * WC          # 240
NBX = W // TW           # 12 block cols
NBY_BAND = 2
BLOCKS_BAND = NBX * NBY_BAND  # 24
NBANDS = 3
SLAB_ROWS = 40
SLAB_FREE = SLAB_ROWS * W   # 3840
TEMP = 0.07 + 1e-10
LN_INV_T = -math.log(TEMP)
NEG = -1e30

# band i: z1 rows [32i, 32i+32); z2 loaded rows [z2y0, z2y0+36);
# slab row of global y = y - slab_row0 ; data col = (slabrow+1)*96 + x
# (the +1 guard row absorbs the woff-2 underflow of bx=0 blocks)
_Z2Y0 = (0, 30, 60)
_SLAB_ROW0 = (-2, 30, 60)
# memset these (start, size) elem ranges of each slab; DMA fills the rest
_MEMSET = (
    ((0, 3 * W), (39 * W, W)),          # guard row + 2 ghost rows ; pad row
    ((0, W), (37 * W, 3 * W)),          # guard row ; rows 36,37 unused + pad
    ((0, W), (37 * W, 3 * W)),          # guard row ; 2 ghost rows + pad
)
_LOAD_COL0 = (3 * W, W, W)              # col of first loaded pixel

NT_Z2 = 27              # z2 tiles of 128 pixels per band (27*128 = 3456 px)
Z2G = 4                 # tiles per z2 load group


def _sub(ap, off, dims):
    """AP at element offset `off` into `ap`'s free space with free dims `dims`."""
    return bass.AP(ap.tensor, ap.offset + off, [list(ap.ap[0])] + [list(d) for d in dims])


def _split_pe_waits(nc):
    """Walrus codegen accepts at most one sync-wait per compute-engine
    instruction: hoist excess waits onto wait-carrier NoOps."""
    n = 0
    for f in nc.m.functions:
        for blk in f.blocks:
            out = []
            for ins in blk.instructions:
                si = ins.sync_info
                if si is not None and si.on_wait and len(si.on_wait) > 1:
                    for w in si.on_wait[:-1]:
                        n += 1
                        out.append(mybir.InstNoOp(
                            name=f"EVW-{n}-{ins.name}",
                            engine=ins.engine,
                            text_hint="wait_carrier",
                            bass_nofuse=True,
                            ins=[], outs=[],
                            sync_info=mybir.SyncInfo(on_wait=[w], on_update=[]),
                        ))
                    ins.sync_info = mybir.SyncInfo(
                        on_wait=[si.on_wait[-1]], on_update=si.on_update)
                out.append(ins)
            blk.instructions = out


def _emit(ctx: ExitStack, tc: tile.TileContext, z1, z2, w1, w2, out):
    nc = tc.nc
    KST = int(os.environ.get("KST", "9"))
    CAST = int(os.environ.get("KCAST", "1"))
    PS32 = int(os.environ.get("KPS32", "0"))
    KSQ = int(os.environ.get("KSQ", "1"))
    KGD = int(os.environ.get("KGD", "1"))

    const = ctx.enter_context(tc.tile_pool(name="const", bufs=1))
    btmp_ctx = ExitStack()
    btmp = btmp_ctx.enter_context(tc.tile_pool(name="btmp", bufs=1))

    # ---------------- constants ----------------
    jj_i = btmp.tile([128, 128], I32)
    nc.gpsimd.iota(jj_i, pattern=[[1, 128]], base=0, channel_multiplier=0)
    jj_f = btmp.tile([128, 128], F32)
    nc.vector.tensor_copy(jj_f, jj_i)
    pv0 = btmp.tile([128, 1], I32)
    nc.gpsimd.iota(pv0, pattern=[[1, 1]], base=0, channel_multiplier=1)
    pv0f = btmp.tile([128, 1], F32)
    nc.vector.tensor_copy(pv0f, pv0)
    ident32 = const.tile([128, 128], F32)
    nc.vector.tensor_scalar(ident32, jj_f, pv0f, None, op0=ALU.is_equal)
    ident_bf = const.tile([128, 128], BF16)
    nc.vector.tensor_copy(ident_bf, ident32)

    ones_c = const.tile([128, 1], F32)
    nc.vector.memset(ones_c, 1.0)
    zsrc = const.tile([128, 3 * W], F32)
    nc.vector.memset(zsrc, 0.0)
    onesrc = const.tile([128, 3 * W], F32)
    nc.vector.memset(onesrc, 1.0)
    ones_r = const.tile([1, 128], F32)
    nc.vector.memset(ones_r, 1.0)
    zb_bf = const.tile([128, 1], BF16)
    nc.vector.memset(zb_bf, 0.0)
    warm = const.tile([128, 1], BF16)
    nc.scalar.activation(warm, zb_bf, ACTF.Square, bias=zb_bf)

    # band masks over the 20x12 window: in-band iff (0<=ny-ty<5)(0<=nx-tx<5)
    # (ty=p>>3, tx=p&7). maskc_* = 0 in-band else -1e30 (added to the Gram);
    # m_* = 1 in-band else 0 (bf16, for the A reduce).
    tyv_i = btmp.tile([128, 1], I32)
    nc.vector.tensor_scalar(tyv_i, pv0, 3, None, op0=ALU.logical_shift_right)
    txv_i = btmp.tile([128, 1], I32)
    nc.vector.tensor_scalar(txv_i, pv0, 7, None, op0=ALU.bitwise_and)
    tyv = btmp.tile([128, 1], F32)
    nc.vector.tensor_copy(tyv, tyv_i)
    txv = btmp.tile([128, 1], F32)
    nc.vector.tensor_copy(txv, txv_i)

    nyt_i = btmp.tile([128, NWIN], I32)
    nc.gpsimd.iota(nyt_i.rearrange("p (a b) -> p a b", b=WC),
                   pattern=[[1, WR], [0, WC]], base=0, channel_multiplier=0)
    nxt_i = btmp.tile([128, NWIN], I32)
    nc.gpsimd.iota(nxt_i.rearrange("p (a b) -> p a b", b=WC),
                   pattern=[[0, WR], [1, WC]], base=0, channel_multiplier=0)
    nyt = btmp.tile([128, NWIN], F32)
    nc.vector.tensor_copy(nyt, nyt_i)
    nxt = btmp.tile([128, NWIN], F32)
    nc.vector.tensor_copy(nxt, nxt_i)

    dyt = btmp.tile([128, NWIN], F32)
    nc.vector.tensor_scalar(dyt, nyt, tyv, None, op0=ALU.subtract)
    dxt = btmp.tile([128, NWIN], F32)
    nc.vector.tensor_scalar(dxt, nxt, txv, None, op0=ALU.subtract)

    tmp_a = btmp.tile([128, NWIN], F32)
    tmp_b = btmp.tile([128, NWIN], F32)
    mf = btmp.tile([128, NWIN], F32)
    nc.vector.tensor_scalar(tmp_a, dyt, 0.0, None, op0=ALU.is_ge)
    nc.vector.tensor_scalar(tmp_b, dyt, 5.0, None, op0=ALU.is_lt)
    nc.vector.tensor_mul(mf, tmp_a, tmp_b)
    nc.vector.tensor_scalar(tmp_a, dxt, 0.0, None, op0=ALU.is_ge)
    nc.vector.tensor_mul(mf, mf, tmp_a)
    nc.vector.tensor_scalar(tmp_b, dxt, 5.0, None, op0=ALU.is_lt)
    nc.vector.tensor_mul(mf, mf, tmp_b)

    mf_l = btmp.tile([128, NWIN], F32)
    nc.vector.tensor_scalar(tmp_a, nxt, 2.0, None, op0=ALU.is_ge)
    nc.vector.tensor_mul(mf_l, mf, tmp_a)
    mf_r = btmp.tile([128, NWIN], F32)
    nc.vector.tensor_scalar(tmp_b, nxt, 9.0, None, op0=ALU.is_le)
    nc.vector.tensor_mul(mf_r, mf, tmp_b)

    maskc_int = const.tile([128, NWIN], BF16)
    maskc_left = const.tile([128, NWIN], BF16)
    maskc_right = const.tile([128, NWIN], BF16)
    for mcb, src in ((maskc_int, mf), (maskc_left, mf_l), (maskc_right, mf_r)):
        # maskc = (m - 1) * 1e30  -> 0 in-band, -1e30 out
        nc.vector.tensor_scalar(tmp_a, src, -1.0, -NEG, op0=ALU.add, op1=ALU.mult)
        nc.vector.tensor_copy(mcb, tmp_a)

    # B compensation: +1 per out-of-image (dy,dx) the x-masks removed
    cl_f = btmp.tile([128, 1], F32)
    nc.vector.tensor_scalar(cl_f, txv, -1.0, 2.0, op0=ALU.mult, op1=ALU.add)  # 2-tx
    c_left = btmp.tile([128, 1], F32)
    nc.vector.tensor_scalar(c_left, cl_f, 0.0, 5.0, op0=ALU.max, op1=ALU.mult)
    cr_f = btmp.tile([128, 1], F32)
    nc.vector.tensor_scalar(cr_f, txv, 5.0, None, op0=ALU.subtract)           # tx-5
    c_right = btmp.tile([128, 1], F32)
    nc.vector.tensor_scalar(c_right, cr_f, 0.0, 5.0, op0=ALU.max, op1=ALU.mult)

    ctile = const.tile([128, BLOCKS_BAND], F32)
    nc.vector.memset(ctile, 0.0)
    for cc, src in ((0, c_left), (12, c_left), (11, c_right), (23, c_right)):
        nc.vector.tensor_copy(ctile[:, cc:cc + 1], src)

    btmp_ctx.close()

    CAST0 = int(os.environ.get("KCAST", "1"))
    cols = ctx.enter_context(tc.tile_pool(name="cols", bufs=1))
    colsb = ctx.enter_context(tc.tile_pool(name="colsb", bufs=2))
    slabp = ctx.enter_context(tc.tile_pool(name="slab", bufs=2 if CAST0 else 1))
    z2stage = ctx.enter_context(tc.tile_pool(name="z2stage", bufs=4 if CAST0 else 2))
    z1stage = ctx.enter_context(tc.tile_pool(name="z1stage", bufs=8 if CAST0 else 4))
    sqscr = ctx.enter_context(tc.tile_pool(name="sqscr", bufs=2))
    z1tp = ctx.enter_context(tc.tile_pool(name="z1tp", bufs=1))
    gstage = ctx.enter_context(tc.tile_pool(name="gstage", bufs=4))
    psum_tp = ctx.enter_context(tc.tile_pool(name="psum_tp", bufs=2, space="PSUM"))
    psum_g = ctx.enter_context(tc.tile_pool(name="psum_g", bufs=3, space="PSUM"))
    psum_s = ctx.enter_context(tc.tile_pool(name="psum_s", bufs=1, space="PSUM"))

    # ---------------- w_avg ----------------
    wt1 = const.tile([128, 72], F32)
    wt2 = const.tile([128, 72], F32)
    for wt, wsrc in ((wt1, w1), (wt2, w2)):
        nc.sync.dma_start(out=wt, in_=wsrc)
    wbar = const.tile([128, 72], F32)
    winv = const.tile([128, 2], F32)
    for j, wt in enumerate((wt1, wt2)):
        wsum = cols.tile([128, 1], F32, name=f"wsum{j}")
        nc.vector.reduce_sum(wsum, wt, axis=mybir.AxisListType.X)
        pswt = psum_s.tile([1, 1], F32, name=f"pswt{j}", tag="s")
        nc.tensor.matmul(pswt, lhsT=wsum, rhs=ones_c, start=True, stop=True)
        stot = cols.tile([1, 1], F32, name=f"stot{j}")
        nc.scalar.copy(stot, pswt)
        nc.vector.tensor_scalar(stot, stot, 1e-10, None, op0=ALU.add)
        nc.vector.reciprocal(stot, stot)
        psb = psum_s.tile([128, 1], F32, name=f"psb{j}", tag="s")
        nc.tensor.matmul(psb, lhsT=ones_r, rhs=stot, start=True, stop=True)
        nc.scalar.copy(winv[:, j:j + 1], psb)
    t_a = const.tile([128, 72], F32)
    nc.vector.tensor_scalar_mul(t_a, wt1, winv[:, 0:1])
    t_b = const.tile([128, 72], F32)
    nc.vector.tensor_scalar_mul(t_b, wt2, winv[:, 1:2])
    nc.vector.tensor_add(wbar, t_a, t_b)
    nc.vector.tensor_scalar_mul(wbar, wbar, 0.5)

    alignB = cols.tile([128, NBANDS], F32)
    posB = cols.tile([128, NBANDS], F32)
    nc.vector.memset(alignB, 0.0)
    nc.vector.memset(posB, 0.0)

    # ---------------- band state ----------------
    state = {}

    def emit_rsqrt(ss, y, n, tag, inv_scale=1.0):
        """y[:, :n] = inv_scale / sqrt(ss[:, :n]) via quake seed + 3 Newton
        iterations, all on DVE (no ScalarE act-table traffic)."""
        qi = colsb.tile([128, ss.shape[1]], I32, name=f"q_{tag}", tag=f"q_{tag}")
        nc.vector.tensor_scalar(qi[:, :n], ss[:, :n].bitcast(I32), 1, None,
                                op0=ALU.logical_shift_right)
        nc.vector.tensor_scalar(qi[:, :n], qi[:, :n], -1, None,
                                op0=ALU.bitwise_xor)
        nc.vector.tensor_scalar(qi[:, :n], qi[:, :n], 0x5F3759E0, None,
                                op0=ALU.add)
        yv = y[:, :n]
        t1 = colsb.tile([128, ss.shape[1]], F32, name=f"t1_{tag}", tag=f"t1_{tag}")
        t2 = colsb.tile([128, ss.shape[1]], F32, name=f"t2_{tag}", tag=f"t2_{tag}")
        cur = qi.bitcast(F32)[:, :n]
        for it in range(2):
            last = it == 1
            nc.vector.tensor_mul(t1[:, :n], ss[:, :n], cur)
            nc.vector.tensor_mul(t2[:, :n], t1[:, :n], cur)
            sc = inv_scale if last else 1.0
            nc.vector.tensor_scalar(t2[:, :n], t2[:, :n], -0.5 * sc, 1.5 * sc,
                                    op0=ALU.mult, op1=ALU.add)
            nc.vector.tensor_mul(yv, cur, t2[:, :n])
            cur = yv
        return y

    def emit_A_start(band):
        """Allocate band tiles, memset slab ghosts."""
        z2y0 = _Z2Y0[band]
        slab6 = slabp.tile([128, KC * SLAB_FREE], BF16, name=f"slab6_{band}",
                           tag="slab6")
        r2rep = slabp.tile([128, SLAB_FREE], BF16, name=f"r2rep{band}", tag="r2rep")
        r2band = cols.tile([1, NT_Z2 * 128], BF16, name=f"r2band{band}", tag="r2band")
        ss2 = colsb.tile([128, NT_Z2], F32, name=f"ss2_{band}", tag="ss2")
        ss1 = colsb.tile([128, BLOCKS_BAND], F32, name=f"ss1_{band}", tag="ss1")
        for rng in _MEMSET[band]:
            nc.gpsimd.dma_start(
                out=_sub(slab6, rng[0], [[SLAB_FREE, KC], [1, rng[1]]]),
                in_=bass.AP(zsrc.tensor, zsrc.offset,
                            [list(zsrc.ap[0]), [0, KC], [1, rng[1]]]))
            # ghost/guard cols of r2rep must be 1.0: in-band ghost slots need
            # g2 = 0*1 = 0 (e=1, matching zero-padding), out-of-band ghost
            # slots need g2 = -1e30*1 so e underflows to 0.
            nc.gpsimd.dma_start(
                out=r2rep[:, rng[0]:rng[0] + rng[1]],
                in_=onesrc[:, 0:rng[1]])
        st = dict(slab6=slab6, r2rep=r2rep, r2band=r2band, ss2=ss2, ss1=ss1,
                  z2y0=z2y0, slab_row0=_SLAB_ROW0[band],
                  z1T=[None] * BLOCKS_BAND, r1T=None, acol=None, bcol=None)
        state[band] = st
        return st

    def emit_z2_group(band, grp):
        """Load z2 tiles (cast to bf16), square, transpose into the slab."""
        st = state[band]
        t0, gsz = grp
        gi = t0
        z2y0 = st["z2y0"]
        zt = z2stage.tile([128, gsz * D], BF16, name=f"z2t{band}_{gi}", tag="z2t")
        r0 = z2y0 * W + t0 * 128
        if CAST and KGD:
            # one casting SWDGE DMA for the whole contiguous group
            nc.gpsimd.dma_start(
                out=zt.rearrange("p (t c) -> p t c", c=D),
                in_=bass.AP(z2.tensor, r0 * D,
                            [[D, 128], [128 * D, gsz], [1, D]]))
        elif CAST:
            for j in range(gsz):
                nc.gpsimd.dma_start(
                    out=zt[:, j * D:(j + 1) * D],
                    in_=bass.AP(z2.tensor, (r0 + j * 128) * D,
                                [[D, 128], [1, D]]))
        else:
            ztf = z2stage.tile([128, gsz * D], F32, name=f"z2f{band}_{gi}",
                               tag="z2f")
            st["_ztf"] = ztf
            nc.sync.dma_start(
                out=ztf.rearrange("p (t c) -> p t c", c=D),
                in_=bass.AP(z2.tensor, r0 * D,
                            [[D, 128], [128 * D, gsz], [1, D]]))
            nc.vector.tensor_copy(zt, ztf)
        if PS32:
            assert not CAST
        dst = _LOAD_COL0[band] + t0 * 128
        for j in range(gsz if KSQ else 0):
            t = t0 + j
            if t % 3 == 0 or t == 26:
                continue  # norm via PE diag below (after slab copies)
            scr = sqscr.tile([128, D], BF16, name=f"z2sq{band}_{t}", tag="zsq")
            nc.scalar.activation(scr, zt[:, j * D:(j + 1) * D], ACTF.Square,
                                 bias=zb_bf, accum_out=st["ss2"][:, t:t + 1])
        # transposes into slab6
        for k in range(KC):
            if PS32:
                ps = psum_tp.tile([128, Z2G * 128], F32,
                                  name=f"z2ps{band}_{gi}_{k}", tag="tp")
                for j in range(gsz):
                    nc.tensor.transpose(
                        ps[:, j * 128:(j + 1) * 128],
                        ztf[:, j * D + k * 128:j * D + (k + 1) * 128],
                        ident32)
            else:
                ps = psum_tp.tile([128, Z2G * 128], BF16,
                                  name=f"z2ps{band}_{gi}_{k}", tag="tp")
                for j in range(gsz):
                    nc.tensor.transpose(
                        ps[:, j * 128:(j + 1) * 128],
                        zt[:, j * D + k * 128:j * D + (k + 1) * 128],
                        ident_bf)
            nc.vector.tensor_copy(
                _sub(st["slab6"], k * SLAB_FREE + dst, [[1, gsz * 128]]),
                ps[:, :gsz * 128])
        # z2 norms for selected tiles: PE self-matmul Gram diagonal
        for j in range(gsz if KSQ else 0):
            t = t0 + j
            if not (t % 3 == 0 or t == 26):
                continue
            sg = psum_s.tile([128, 128], F32, name=f"sg{band}_{t}", tag="sg",
                             bufs=1)
            for k in range(KC):
                cols = _sub(st["slab6"], k * SLAB_FREE + dst + j * 128,
                            [[1, 128]])
                nc.tensor.matmul(sg, lhsT=cols, rhs=cols,
                                 start=(k == 0), stop=(k == KC - 1))
            dscr = gstage.tile([128, 128], F32, name=f"dsc{band}_{t}",
                               tag="dscr", bufs=2)
            nc.vector.affine_mul_reduce(
                out=dscr, accum_out=st["ss2"][:, t:t + 1],
                in0=sg, in1=ident32, scale=1.0, bias=0.0)

    def emit_z1_block_load(band, bb):
        """Load z1 block (cast), square (DVE), transpose into z1T."""
        st = state[band]
        byl, bx = divmod(bb, NBX)
        y0 = 32 * band + TH * byl
        z1t = z1stage.tile([128, D], BF16, name=f"z1t{band}_{bb}", tag="z1t")
        if CAST:
            nc.gpsimd.dma_start(
                out=z1t,
                in_=bass.AP(z1.tensor, (y0 * W + bx * TW) * D,
                            [[W * D, TH], [D, TW], [1, D]]))
        else:
            z1f = z1stage.tile([128, D], F32, name=f"z1f{band}_{bb}", tag="z1f")
            nc.sync.dma_start(
                out=z1f,
                in_=bass.AP(z1.tensor, (y0 * W + bx * TW) * D,
                            [[W * D, TH], [D, TW], [1, D]]))
            nc.vector.tensor_copy(z1t, z1f)
        if KSQ and bb % 8 != 0:
            scr = sqscr.tile([128, D], BF16, name=f"z1sq{band}_{bb}", tag="zsq")
            nc.scalar.activation(scr, z1t, ACTF.Square, bias=zb_bf,
                                 accum_out=st["ss1"][:, bb:bb + 1])
        z1T = z1tp.tile([128, D], BF16, name=f"z1T{band}_{bb}", tag=f"z1T{bb}")
        tsrc, tid = (z1f, ident32) if PS32 else (z1t, ident_bf)
        tdt = F32 if PS32 else BF16
        psa = psum_tp.tile([128, D], tdt, name=f"z1psa{band}_{bb}", tag="tpz", bufs=1)
        for k in range(KC):
            nc.tensor.transpose(psa[:, k * 128:(k + 1) * 128],
                                tsrc[:, k * 128:(k + 1) * 128], tid)
        nc.vector.tensor_copy(z1T, psa)
        st["z1T"][bb] = z1T
        if KSQ and bb % 8 == 0:
            sg = psum_s.tile([128, 128], F32, name=f"sg1_{band}_{bb}", tag="sg",
                             bufs=1)
            for k in range(KC):
                nc.tensor.matmul(sg, lhsT=z1T[:, k * 128:(k + 1) * 128],
                                 rhs=z1T[:, k * 128:(k + 1) * 128],
                                 start=(k == 0), stop=(k == KC - 1))
            dscr = gstage.tile([128, 128], F32, name=f"dsc1_{band}_{bb}",
                               tag="dscr", bufs=2)
            nc.vector.affine_mul_reduce(
                out=dscr, accum_out=st["ss1"][:, bb:bb + 1],
                in0=sg, in1=ident32, scale=1.0, bias=0.0)

    def emit_r2(band):
        """Band-level r2 = 1/||z2|| -> r2band row -> broadcast into r2rep."""
        if KST < 2:
            return
        st = state[band]
        r2f = colsb.tile([128, NT_Z2], F32, name=f"r2f{band}", tag="r2f")
        emit_rsqrt(st["ss2"], r2f, NT_Z2, "r2")
        r2c = colsb.tile([128, NT_Z2], BF16, name=f"r2c_{band}", tag="r2c")
        nc.vector.tensor_copy(r2c, r2f)
        psr = psum_s.tile([NT_Z2, 128], BF16, name=f"psr2_{band}", tag="s")
        nc.tensor.transpose(psr, r2c, ident_bf)
        r2t = cols.tile([NT_Z2, 128], BF16, name=f"r2t{band}", tag="r2t", bufs=2)
        nc.vector.tensor_copy(r2t, psr)
        nc.sync.dma_start(
            out=_sub(st["r2band"], 0, [[1, NT_Z2 * 128]]), in_=r2t)
        nc.gpsimd.partition_broadcast(
            _sub(st["r2rep"], _LOAD_COL0[band], [[1, NT_Z2 * 128]]),
            _sub(st["r2band"], 0, [[1, NT_Z2 * 128]]))

    def emit_B(band):
        """Batched r1 = 1/(||z1||*T) via DVE quake rsqrt."""
        st = state[band]
        r1T = colsb.tile([128, BLOCKS_BAND], F32, name=f"r1T{band}", tag="r1T")
        if KST >= 2:
            emit_rsqrt(st["ss1"], r1T, BLOCKS_BAND, "r1", inv_scale=1.0 / TEMP)
        else:
            nc.vector.memset(r1T, 1.0)
        r1b = colsb.tile([128, BLOCKS_BAND], BF16, name=f"r1b{band}", tag="r1b")
        nc.vector.tensor_copy(r1b, r1T)
        st["r1T"] = r1T
        st["r1b"] = r1b
        st["acol"] = colsb.tile([128, BLOCKS_BAND], F32, name=f"acol{band}", tag="acol")
        st["bcol"] = colsb.tile([128, BLOCKS_BAND], F32, name=f"bcol{band}", tag="bcol")

    def emit_C_block(band, bb):
        st = state[band]
        byl, bx = divmod(bb, NBX)
        y0 = 32 * band + TH * byl
        s0 = (y0 - 2) - st["slab_row0"]
        woff = (s0 + 1) * W + TW * bx - 2
        maskc = maskc_left if bx == 0 else (maskc_right if bx == NBX - 1 else maskc_int)

        if KST < 3:
            return
        g = psum_g.tile([128, NWIN], F32, name=f"g{band}_{bb}", tag="g")
        nc.tensor.matmul(g, lhsT=ident_bf, rhs=maskc, start=True, stop=False)
        z1T = st["z1T"][bb]
        for k in range(KC):
            nc.tensor.matmul(
                g,
                lhsT=z1T[:, k * 128:(k + 1) * 128],
                rhs=_sub(st["slab6"], k * SLAB_FREE + woff, [[W, WR], [1, WC]]),
                start=False, stop=(k == KC - 1))

        if KST < 4:
            return
        g2 = gstage.tile([128, NWIN], BF16, name=f"g2_{band}_{bb}", tag="g2")
        nc.vector.tensor_mul(g2, g, _sub(st["r2rep"], woff, [[W, WR], [1, WC]]))
        e_t = gstage.tile([128, NWIN], BF16, name=f"e{band}_{bb}", tag="e")
        nc.scalar.activation(e_t, g2, ACTF.Exp, bias=zb_bf,
                             scale=st["r1T"][:, bb:bb + 1],
                             accum_out=st["bcol"][:, bb:bb + 1])
        scr2 = gstage.tile([128, NWIN], BF16, name=f"tt2s{band}_{bb}", tag="tt2s")
        nc.vector.affine_mul_reduce(
            out=scr2, accum_out=st["acol"][:, bb:bb + 1],
            in0=g2, in1=e_t, scale=1.0, bias=0.0)

    def emit_tail(band):
        if KST < 5:
            return
        st = state[band]
        bcol, acol, r1T = st["bcol"], st["acol"], st["r1T"]
        nc.vector.tensor_add(bcol, bcol, ctile)
        rB = colsb.tile([128, BLOCKS_BAND], F32, name=f"rB{band}", tag="rB")
        nc.vector.reciprocal(rB, bcol)
        sstar = colsb.tile([128, BLOCKS_BAND], F32, name=f"sstar{band}", tag="sstar")
        nc.vector.tensor_mul(sstar, acol, r1T)
        nc.vector.tensor_mul(sstar, sstar, rB)
        nc.vector.tensor_scalar_mul(sstar, sstar, TEMP)

        u = colsb.tile([128, BLOCKS_BAND], F32, name=f"u{band}", tag="u")
        nc.vector.tensor_scalar(u, sstar, -1.0, 1.0, op0=ALU.mult, op1=ALU.add)
        uw = colsb.tile([128, BLOCKS_BAND], F32, name=f"uw{band}", tag="uw")
        nc.vector.affine_mul_reduce(
            out=uw, accum_out=alignB[:, band:band + 1],
            in0=u, in1=wbar[:, band * 24:(band + 1) * 24],
            scale=1.0, bias=0.0)
        nc.vector.reduce_sum(posB[:, band:band + 1], sstar,
                             axis=mybir.AxisListType.X)

    # ---------------- schedule ----------------
    # A-items per band: 7 z2 groups + 24 z1 block loads, interleaved so DMA
    # and the square/transpose engines stay fed. Bands are software-pipelined:
    # band b+1's A-items are emitted between band b's C-blocks.
    def a_items(band):
        sizes = (4, 4, 4, 4, 4, 4, 3)
        items = []
        bi = 0
        t0 = 0
        for gi, gsz in enumerate(sizes):
            items.append(("g", (t0, gsz)))
            t0 += gsz
            if gi == len(sizes) - 1:
                items.append(("r2", 0))
            for _ in range(3):
                if bi < BLOCKS_BAND:
                    items.append(("b", bi))
                    bi += 1
        while bi < BLOCKS_BAND:
            items.append(("b", bi))
            bi += 1
        return items

    def emit_a_item(band, it):
        if it[0] == "g":
            emit_z2_group(band, it[1])
        elif it[0] == "r2":
            emit_r2(band)
        else:
            emit_z1_block_load(band, it[1])

    emit_A_start(0)
    for it in a_items(0):
        emit_a_item(0, it)
    emit_B(0)
    for band in range(NBANDS):
        nxt = band + 1
        if nxt < NBANDS:
            emit_A_start(nxt)
            items = a_items(nxt)
            # ~31 A-items over 24 C-blocks
            k = 0
            for bb in range(BLOCKS_BAND):
                emit_C_block(band, bb)
                want = ((bb + 1) * len(items) + BLOCKS_BAND - 1) // BLOCKS_BAND
                while k < min(want, len(items)):
                    emit_a_item(nxt, items[k])
                    k += 1
            while k < len(items):
                emit_a_item(nxt, items[k])
                k += 1
            emit_tail(band)
            emit_B(nxt)
        else:
            for bb in range(BLOCKS_BAND):
                emit_C_block(band, bb)
            emit_tail(band)

    # ---------------- final scalars ----------------
    if KST < 5:
        nc.vector.memset(alignB, 0.0)
        nc.vector.memset(posB, 0.0)
    fin = cols.tile([128, 2], F32)
    nc.vector.reduce_sum(fin[:, 0:1], alignB, axis=mybir.AxisListType.X)
    nc.vector.reduce_sum(fin[:, 1:2], posB, axis=mybir.AxisListType.X)
    psf = psum_s.tile([2, 1], F32, tag="s")
    nc.tensor.matmul(psf, lhsT=fin, rhs=ones_c, start=True, stop=True)
    fsb = cols.tile([2, 1], F32)
    nc.scalar.copy(fsb, psf)
    nc.sync.dma_start(out=out[:, :], in_=fsb)


def build_nc():
    nc = bacc.Bacc("TRN2", target_bir_lowering=False, debug=False)
    z1 = nc.dram_tensor("z1", [NPIX, D], F32, kind="ExternalInput")
    z2 = nc.dram_tensor("z2", [NPIX, D], F32, kind="ExternalInput")
    w1 = nc.dram_tensor("w1", [128, 72], F32, kind="ExternalInput")
    w2 = nc.dram_tensor("w2", [128, 72], F32, kind="ExternalInput")
    out = nc.dram_tensor("out", [2, 1], F32, kind="ExternalOutput")
    with tile.TileContext(nc) as tc:
        with ExitStack() as ctx:
            _emit(ctx, tc, z1.ap(), z2.ap(), w1.ap(), w2.ap(), out.ap())
    nc.compile()
    return nc


_NC_CACHE = None


def _get_nc():
    global _NC_CACHE
    if _NC_CACHE is None:
        _NC_CACHE = build_nc()
    return _NC_CACHE


def make_in_maps(z1, z2, w1, w2):
    z1 = np.asarray(z1, dtype=np.float32)
    z2 = np.asarray(z2, dtype=np.float32)

    def _wblk(w):
        w = np.asarray(w, dtype=np.float32).reshape(z1.shape[0], 6, TH, NBX, TW)
        return np.ascontiguousarray(
            w.transpose(0, 2, 4, 1, 3).reshape(z1.shape[0], 128, 72))
    w1 = _wblk(w1)
    w2 = _wblk(w2)
    return [
        {
            "z1": np.ascontiguousarray(z1[b]),
            "z2": np.ascontiguousarray(z2[b]),
            "w1": np.ascontiguousarray(w1[b]),
            "w2": np.ascontiguousarray(w2[b]),
        }
        for b in range(z1.shape[0])
    ]


def combine_outputs(outs):
    a = np.stack([o.reshape(2) for o in outs])  # [B, 2]
    align = a[:, 0].mean()
    pos = (a[:, 1] / NPIX).mean()
    return np.stack([align, pos]).astype(np.float32)


def kernel(z1, z2, w1, w2, H=None, W=None, **_unused):
    from concourse.bass_utils import run_bass_kernel_spmd

    nc = _get_nc()
    in_maps = make_in_maps(z1, z2, w1, w2)
    res = run_bass_kernel_spmd(nc, in_maps, core_ids=list(range(len(in_maps))))
    return combine_outputs([r["out"] for r in res.results])
